# revision 11
# baseline (speedup 1.0000x reference)
"""v4: eT layout — e on partitions for the enc matmul epilogue.

Per core (8 batches x 2048 rows):
  - only ONE h load: hT chunks (d on partitions) via DMA-xbar transpose of
    host-cast fp16 h  [4 chunks x (128 x 2048) per batch]
  - enc: encT[c] (128e x 512l) = sum_d Wh[d][c].T @ hT[d]
    weight-reuse ordering: fix (e-chunk c, half), loop d, stream 2 l-blocks
  - cov term: K=1 matmul  lhsT=Wc[c] (1x128), rhs=cov16 row (1x512)
  - dec+bias: folded into tanh's per-partition bias column (e on partitions!)
  - scores: M=1 matvec  lhsT=v[c] (128x1), rhs=feats[c] -> psum (1x512)/l-block
  - softmax on (1x2048) rows; exp's accum_out gives Z for free
  - a broadcast across partitions via ones-trick matmul (f16 psum out)
  - ctx: DVE scalar_tensor_tensor accum: sum_l a_rep*hT -> (128x1) per (d,lb)
  - ctxT gathered (128 x 8b*4c), one final transpose -> clean DMA out
"""

import numpy as np

B, L, D = 64, 2048, 512
NCORES = 8
BPC = B // NCORES
NLB = L // 512  # 4 l-blocks per batch

_STATE = {}


def _build_bass(repeat=1):
    import concourse.tile as tile
    from concourse import bacc, mybir

    f32 = mybir.dt.float32
    f16 = mybir.dt.float16
    Alu = mybir.AluOpType
    Act = mybir.ActivationFunctionType

    nc = bacc.Bacc("TRN2", target_bir_lowering=False, debug=False, num_devices=NCORES)

    h_d = nc.dram_tensor("h16", [BPC, L, D], f16, kind="ExternalInput").ap()
    wh_d = nc.dram_tensor("wh", [D, D], f16, kind="ExternalInput").ap()       # [d, e]
    cov_d = nc.dram_tensor("cov16", [1, BPC, L], f16, kind="ExternalInput").ap()
    wcr_d = nc.dram_tensor("wcr", [1, 4, 128], f16, kind="ExternalInput").ap()
    bcol_d = nc.dram_tensor("bcol", [128, BPC * 4], f32, kind="ExternalInput").ap()
    vcol_d = nc.dram_tensor("vcol", [128, 4], f16, kind="ExternalInput").ap()
    ones16_d = nc.dram_tensor("ones16", [1, 128], f16, kind="ExternalInput").ap()
    id32_d = nc.dram_tensor("id32", [128, 128], f32, kind="ExternalInput").ap()

    a_out = nc.dram_tensor("a_out", [BPC, L], f32, kind="ExternalOutput").ap()
    ctx_out = nc.dram_tensor("ctx_out", [BPC, D], f32, kind="ExternalOutput").ap()

    from contextlib import ExitStack

    with tile.TileContext(nc) as tc, ExitStack() as stk:
        consts = stk.enter_context(tc.tile_pool(name="consts", bufs=1))
        hT_pool = stk.enter_context(tc.tile_pool(name="hT", bufs=10))
        ft_pool = stk.enter_context(tc.tile_pool(name="ft", bufs=16))
        arep_pool = stk.enter_context(tc.tile_pool(name="arep", bufs=5))
        scr_pool = stk.enter_context(tc.tile_pool(name="scr", bufs=4))
        rows_pool = stk.enter_context(tc.tile_pool(name="rows", bufs=2))
        small_pool = stk.enter_context(tc.tile_pool(name="small", bufs=4))
        persist = stk.enter_context(tc.tile_pool(name="persist", bufs=1))
        pu_pool = stk.enter_context(tc.tile_pool(name="pu", bufs=3, space="PSUM"))
        ps_pool = stk.enter_context(tc.tile_pool(name="ps", bufs=1, space="PSUM"))
        pz_pool = stk.enter_context(tc.tile_pool(name="pz", bufs=1, space="PSUM"))
        if True:
            wh_sb = consts.tile([128, 4, D], f16, tag="wh")  # [d_part, d_chunk, e]
            nc.sync.dma_start(out=wh_sb[:], in_=wh_d.rearrange("(i p) e -> p i e", p=128))
            cov_sb = consts.tile([1, BPC, L], f16, tag="cov")
            nc.sync.dma_start(out=cov_sb[:], in_=cov_d)
            wcr_sb = consts.tile([1, 4, 128], f16, tag="wcr")
            nc.sync.dma_start(out=wcr_sb[:], in_=wcr_d)
            bcol_sb = consts.tile([128, BPC * 4], f32, tag="bcol")
            nc.sync.dma_start(out=bcol_sb[:], in_=bcol_d)
            vcol_sb = consts.tile([128, 4], f16, tag="vcol")
            nc.sync.dma_start(out=vcol_sb[:], in_=vcol_d)
            ones16_sb = consts.tile([1, 128], f16, tag="ones16")
            nc.sync.dma_start(out=ones16_sb[:], in_=ones16_d)
            id32_sb = consts.tile([128, 128], f32, tag="id32")
            nc.sync.dma_start(out=id32_sb[:], in_=id32_d)

            ctxT_all = persist.tile([128, BPC * 4], f32, tag="ctxT")

            rep_ctx = tc.For_i(0, repeat, 1) if repeat > 1 else None
            if rep_ctx is not None:
                stk.enter_context(rep_ctx)

            for b in range(BPC):
                # ---- hT chunks via DMA transpose ----
                hT = []
                for i in range(4):
                    ht = hT_pool.tile([128, L], f16, tag="hT")
                    nc.scalar.dma_start(
                        out=ht[:],
                        in_=h_d[b, :, 128 * i : 128 * (i + 1)],
                        transpose=True,
                    )
                    hT.append(ht)

                # ---- enc + cov + tanh: (e-chunk, lb-pair) passes ----
                ft = [[None, None] for _ in range(4)]  # [c][half]
                for c in range(4):
                    for hf in range(2):
                        u = pu_pool.tile([128, 2, 512], f32, tag="pu")  # 2 banks
                        for d in range(4):
                            for j in range(2):
                                lb = 2 * hf + j
                                nc.tensor.matmul(
                                    u[:, j, :],
                                    lhsT=wh_sb[:, d, 128 * c : 128 * (c + 1)],
                                    rhs=hT[d][:, 512 * lb : 512 * (lb + 1)],
                                    start=(d == 0),
                                    stop=False,
                                )
                        for j in range(2):
                            lb = 2 * hf + j
                            nc.tensor.matmul(
                                u[:, j, :],
                                lhsT=wcr_sb[0:1, c, :],
                                rhs=cov_sb[0:1, b, 512 * lb : 512 * (lb + 1)],
                                start=False,
                                stop=True,
                            )
                        ftc = ft_pool.tile([128, 2, 512], f16, tag="ft")
                        nc.scalar.activation(
                            ftc[:], u[:], Act.Tanh,
                            bias=bcol_sb[:, 4 * b + c : 4 * b + c + 1],
                        )
                        ft[c][hf] = ftc

                # ---- scores ----
                srow = rows_pool.tile([1, L], f32, tag="srow")
                for lb in range(NLB):
                    sps = ps_pool.tile([1, 512], f32, tag="ps")
                    for c in range(4):
                        nc.tensor.matmul(
                            sps[:],
                            lhsT=vcol_sb[:, c : c + 1],
                            rhs=ft[c][lb // 2][:, lb % 2, :],
                            start=(c == 0),
                            stop=(c == 3),
                        )
                    nc.scalar.copy(srow[0:1, 512 * lb : 512 * (lb + 1)], sps[:])

                # ---- softmax on the (1 x 2048) row; Z via exp accum_out ----
                e_row = rows_pool.tile([1, L], f32, tag="erow")
                z = small_pool.tile([1, 1], f32, tag="z")
                nc.scalar.activation(e_row[:], srow[:], Act.Exp, accum_out=z[:])
                zinv = small_pool.tile([1, 1], f32, tag="zinv")
                nc.vector.reciprocal(zinv[:], z[:])
                a32_row = rows_pool.tile([1, L], f32, tag="a32row")
                nc.vector.tensor_scalar(
                    out=a32_row[:], in0=e_row[:], scalar1=zinv[:],
                    scalar2=None, op0=Alu.mult,
                )
                a16_row = rows_pool.tile([1, L], f16, tag="a16row")
                nc.scalar.activation(a16_row[:], e_row[:], Act.Copy, scale=zinv[:])
                nc.sync.dma_start(out=a_out[b : b + 1, :], in_=a32_row[:])

                # ---- broadcast a across partitions (gpsimd, per l-block) ----
                arep = []
                for lb in range(NLB):
                    ar = arep_pool.tile([128, 512], f16, tag="arep")
                    nc.gpsimd.partition_broadcast(
                        ar[:], a16_row[0:1, 512 * lb : 512 * (lb + 1)]
                    )
                    arep.append(ar)

                # ---- ctx: accum_out of a_rep * hT over l ----
                ctx_part = small_pool.tile([128, 4, NLB], f32, tag="ctxpart")
                for i in range(4):
                    for lb in range(NLB):
                        scr = scr_pool.tile([128, 512], f16, tag="scr")
                        nc.vector.scalar_tensor_tensor(
                            out=scr[:],
                            in0=arep[lb][:],
                            scalar=1.0,
                            in1=hT[i][:, 512 * lb : 512 * (lb + 1)],
                            op0=Alu.bypass,
                            op1=Alu.mult,
                            accum_out=ctx_part[:, i, lb : lb + 1],
                        )
                nc.vector.tensor_reduce(
                    ctxT_all[:, 4 * b : 4 * (b + 1)],
                    ctx_part[:],
                    mybir.AxisListType.X,
                    Alu.add,
                )

            # ---- final: transpose ctxT (128 x 32) -> (32 x 128) -> DMA ----
            cT_ps = pz_pool.tile([128, 128], f32, tag="pz")
            nc.tensor.transpose(
                cT_ps[0 : BPC * 4, 0:128], ctxT_all[:], id32_sb[:]
            )
            cT_sb = small_pool.tile([BPC * 4, 128], f32, tag="cTsb")
            nc.scalar.copy(cT_sb[:], cT_ps[0 : BPC * 4, 0:128])
            nc.sync.dma_start(
                out=ctx_out.rearrange("b (c p) -> (b c) p", p=128), in_=cT_sb[:]
            )

    nc.compile()
    return nc


def _get_nc():
    if "nc" not in _STATE:
        _STATE["nc"] = _build_bass()
    return _STATE["nc"]


def _host_prep(h_i, s_t, coverage, Wh, bh, Ws, bs, Wc, bc, Vw):
    base = (s_t @ Ws + bs + bh + bc).astype(np.float32)   # (B, D)
    v = Vw.sum(axis=1)                                    # (D,)
    cov = coverage[..., 0, 0]                             # (B, L)

    h16 = h_i.astype(np.float16)
    cov16 = cov.astype(np.float16)
    wh16 = Wh.astype(np.float16)
    wcr = Wc[0].astype(np.float16).reshape(1, 4, 128)
    vcol = v.astype(np.float16).reshape(4, 128).T.copy()          # (128, 4)
    ones16 = np.ones((1, 128), np.float16)
    id32 = np.eye(128, dtype=np.float32)

    in_maps = []
    for c in range(NCORES):
        sl = slice(c * BPC, (c + 1) * BPC)
        bcol = (
            base[sl].reshape(BPC, 4, 128).transpose(2, 0, 1).reshape(128, BPC * 4)
        ).copy()  # [p, b*4+c]
        in_maps.append(
            {
                "h16": h16[sl],
                "wh": wh16,
                "cov16": cov16[sl].reshape(1, BPC, L),
                "wcr": wcr,
                "bcol": bcol,
                "vcol": vcol,
                "ones16": ones16,
                "id32": id32,
            }
        )
    return in_maps


def kernel(h_i, s_t, coverage, Wh, bh, Ws, bs, Wc, bc, Vw):
    from concourse.bass_utils import run_bass_kernel_spmd

    h_i = np.asarray(h_i, dtype=np.float32)
    coverage = np.asarray(coverage, dtype=np.float32)
    args = dict(
        h_i=h_i,
        s_t=np.asarray(s_t, dtype=np.float32),
        coverage=coverage,
        Wh=np.asarray(Wh, dtype=np.float32),
        bh=np.asarray(bh, dtype=np.float32),
        Ws=np.asarray(Ws, dtype=np.float32),
        bs=np.asarray(bs, dtype=np.float32),
        Wc=np.asarray(Wc, dtype=np.float32),
        bc=np.asarray(bc, dtype=np.float32),
        Vw=np.asarray(Vw, dtype=np.float32),
    )
    in_maps = _host_prep(**args)
    _STATE["in_maps"] = in_maps

    nc = _get_nc()
    res = run_bass_kernel_spmd(nc, in_maps, core_ids=list(range(NCORES)))
    a_t = np.concatenate([res.results[c]["a_out"] for c in range(NCORES)], axis=0)
    ctx = np.concatenate([res.results[c]["ctx_out"] for c in range(NCORES)], axis=0)
    coverage_new = coverage + a_t[:, :, None, None]
    return ctx, a_t, coverage_new
